# revision 1
# baseline (speedup 1.0000x reference)
"""nn_PatchMerging3D Trainium2 kernel.

Full inputs: x (2, 96, 32, 128, 128) f32, w (192, 768), gamma (768), beta (768).
Output: (2, 192, 16, 64, 64) f32.

Sharding: D2 (=16) split across 8 cores, 2 d2-planes per core, both batches.
Per-core slab: x[:, :, 4k:4k+4, :, :].

Per-core kernel (bass/Tile, see work/pm3d.py devnotes):
  * SBUF X layout: partition p = dd*64 + hh*32 + c32 (c = 32g + c32),
    free = g*(GJ*1024) + h2*128 + w. 12 HWDGE DMAs per 4-tile group
    (one per (dd, hh, g)), 3-dim APs, 512B contiguous runs, disjoint
    32-partition groups so all 16 SDMA ports run concurrently.
  * LayerNorm folded into the 768x192 projection algebraically:
      wp = gamma*w ; M = xf @ wp.T   (PE, 6 K=128 chunks, float32r)
      S1 = sum xf (ones column in lhsT, psum row 96)
      S2 = sum xf^2 (ACT Square + ones-lhsT matmuls)
      var = S2/768 - mu^2 + eps ; y = rsig * (M - mu x s + t x sig)
    rank-1 terms accumulated in PSUM by the PE; rsig broadcast across
    partitions via a ones(1,96) matmul.
"""

import os
import sys

for _p in ("/opt/trn_rl_repo", "/root/.axon_site/_ro/trn_rl_repo"):
    if os.path.isdir(_p) and _p not in sys.path:
        sys.path.insert(0, _p)

import numpy as np

import concourse.bacc as bacc
import concourse.mybir as mybir
from concourse.tile import TileContext
from concourse.bass_utils import run_bass_kernel_spmd

F32 = mybir.dt.float32
F32R = mybir.dt.float32r
AF = mybir.ActivationFunctionType
OP = mybir.AluOpType

C = 96
W = 128
W2 = 64
H2T = 8          # h2 values per position tile
NPOS = 512       # positions per tile
KCH = 6          # contraction chunks (g, ww)
GJ = 4           # position tiles per DMA group
LN_EPS = 1e-5
NCORES = 8
PSS_IN_PSB = False   # f32r matmul dst must start at partition 0 (s3d3 ISA)
STAGES = "full"      # "mains" = DMA + main matmuls + evict only (perf probe)
GPSIMD_BCAST = False # rsig broadcast on GpSimd instead of PE matmul + copy

# full-problem per-core loop counts
NB, ND, NJ = 2, 2, 8


def _host_prep(w, gamma, beta):
    w = np.asarray(w, np.float32)
    gamma = np.asarray(gamma, np.float32)
    beta = np.asarray(beta, np.float32)
    wp = w * gamma[None, :]
    s = wp.sum(axis=1)
    t = (w * beta[None, :]).sum(axis=1)

    dd = np.arange(2)[:, None, None]
    hh = np.arange(2)[None, :, None]
    ii = np.arange(32)[None, None, :]
    wA = np.zeros((KCH, 128, 97), np.float32)
    wB = np.zeros((KCH, 128, 96), np.float32)
    for g in range(3):
        for ww in range(2):
            q = g * 2 + ww
            cf = (dd * 384 + hh * 192 + ww * 96 + 32 * g + ii).reshape(128)
            wA[q, :, :96] = wp[0:96, cf].T
            wA[q, :, 96] = 1.0
            wB[q, :, :] = wp[96:192, cf].T
    return {
        "wA": wA,
        "wB": wB,
        "negs": (-s).reshape(1, 192).astype(np.float32),
        "tvec": t.reshape(1, 192).astype(np.float32),
        "ones_row": np.ones((1, 96), np.float32),
    }, bool(np.any(beta != 0.0))


def _tile_body(nc, y, b, dL, j, jl, X, GF,
               wA_sb, wB_sb, negs_sb, tvec_sb, ones_sb, ebias,
               xsq_pool, y_pool, small_pool, rep_pool, psum_pool, psum_s_pool,
               has_beta):
    h0 = H2T * jl
    if STAGES != "mains":
        XQ = xsq_pool.tile([128, 3072], F32R)
        for g in range(3):
            nc.scalar.activation(
                XQ[:, g * 1024:(g + 1) * 1024],
                X[:, g * GF + h0 * 128: g * GF + (h0 + H2T) * 128].bitcast(F32),
                AF.Square)

    psA = psum_pool.tile([97, NPOS], F32)
    if PSS_IN_PSB:
        psBx = psum_pool.tile([97, NPOS], F32)
        psB = psBx[0:96, :]
        psS = psBx[96:97, :]
        psS_tp = (0, 96)
    else:
        psBx = psum_pool.tile([96, NPOS], F32)
        psB = psBx[:]
        psSx = psum_s_pool.tile([1, NPOS], F32)
        psS = psSx[:]
        psS_tp = None
    wA_v = wA_sb[:].rearrange("p (q m) -> p q m", q=KCH)
    wB_v = wB_sb[:].rearrange("p (q m) -> p q m", q=KCH)
    Xv = X[:].rearrange("p (g h2g w2 ww) -> p g h2g w2 ww",
                        g=3, h2g=H2T * GJ, w2=W2, ww=2)
    XQv = (None if STAGES == "mains" else
           XQ[:].rearrange("p (g h2 w2 ww) -> p g h2 w2 ww",
                           g=3, h2=H2T, w2=W2, ww=2))
    for g in range(3):
        for ww in range(2):
            q = g * 2 + ww
            rhs = Xv[:, g, h0:h0 + H2T, :, ww]
            nc.tensor.matmul(psA[:], wA_v[:, q], rhs,
                             start=(q == 0), stop=(q == KCH - 1))
            nc.tensor.matmul(psB, wB_v[:, q], rhs,
                             start=(q == 0), stop=(q == KCH - 1))
            if STAGES != "mains":
                nc.tensor.matmul(psS, wA_v[:, q, 96:97],
                                 XQv[:, g, :, :, ww],
                                 start=(q == 0), stop=(q == KCH - 1),
                                 tile_position=psS_tp, skip_group_check=True)

    if STAGES == "mains":
        yt = y_pool.tile([96, 2 * NPOS], F32)
        nc.vector.tensor_copy(yt[:, 0:NPOS], psA[0:96, :])
        nc.vector.tensor_copy(yt[:, NPOS:], psB)
        nc.sync.dma_start(y[b, dL, j],
                          yt[:].rearrange("p (half f) -> p half f", half=2))
        return
    # stats (DVE-centric; single ACT handoff for sqrt)
    mu_t = small_pool.tile([1, NPOS], F32R)
    sig_t = small_pool.tile([1, NPOS], F32R)
    vm = small_pool.tile([1, 2 * NPOS], F32)
    rsig = small_pool.tile([1, NPOS], F32R)
    mu = mu_t[:]
    sig = sig_t[:]
    var = vm[:, 0:NPOS]
    musq = vm[:, NPOS:]
    nc.vector.tensor_scalar(mu, psA[96:97, :], 1.0 / 768.0, None, OP.mult)
    nc.vector.tensor_tensor(musq, mu.bitcast(F32), mu.bitcast(F32), OP.mult)
    # var = S2/768 - mu^2   (+eps folded into the sqrt bias)
    nc.vector.scalar_tensor_tensor(var, psS, 1.0 / 768.0, musq,
                                   OP.mult, OP.subtract)
    nc.scalar.activation(sig, var, AF.Sqrt, bias=ebias[:])
    with nc.allow_low_precision(reason="f32r rsig for matmul broadcast"):
        nc.vector.reciprocal(rsig[:], sig.bitcast(F32))

    # rank-1 corrections (main group closed; HW has_written still accumulates)
    nc.tensor.matmul(psA[0:96, :], negs_sb[0:1, 0:96], mu,
                     start=False, stop=True, skip_group_check=True)
    nc.tensor.matmul(psB, negs_sb[0:1, 96:192], mu,
                     start=False, stop=True, skip_group_check=True)
    if has_beta:
        nc.tensor.matmul(psA[0:96, :], tvec_sb[0:1, 0:96], sig,
                         start=False, stop=True, skip_group_check=True)
        nc.tensor.matmul(psB, tvec_sb[0:1, 96:192], sig,
                         start=False, stop=True, skip_group_check=True)

    # rsig broadcast across partitions + final scale (DVE)
    rep = rep_pool.tile([96, NPOS], F32)
    if GPSIMD_BCAST:
        nc.gpsimd.partition_broadcast(rep[:], rsig[:].bitcast(F32))
    else:
        psR = psum_s_pool.tile([96, NPOS], F32)
        nc.tensor.matmul(psR[:], ones_sb[:], rsig[:], start=True, stop=True)
        nc.vector.tensor_copy(rep[:], psR[:])
    yt = y_pool.tile([96, 2 * NPOS], F32)
    nc.vector.tensor_tensor(yt[:, 0:NPOS], psA[0:96, :], rep[:], OP.mult)
    nc.vector.tensor_tensor(yt[:, NPOS:], psB, rep[:], OP.mult)

    nc.sync.dma_start(y[b, dL, j],
                      yt[:].rearrange("p (half f) -> p half f", half=2))


def build_kernel(nc, reps=1, has_beta=True):
    x = nc.dram_tensor("x", [NB, C, 2 * ND, 16 * NJ, W], F32,
                       kind="ExternalInput")
    wA_d = nc.dram_tensor("wA", [KCH, 128, 97], F32, kind="ExternalInput")
    wB_d = nc.dram_tensor("wB", [KCH, 128, 96], F32, kind="ExternalInput")
    negs_d = nc.dram_tensor("negs", [1, 192], F32, kind="ExternalInput")
    tvec_d = nc.dram_tensor("tvec", [1, 192], F32, kind="ExternalInput")
    ones_d = nc.dram_tensor("ones_row", [1, 96], F32, kind="ExternalInput")
    y = nc.dram_tensor("y", [NB, ND, NJ, 96, 2, NPOS], F32,
                       kind="ExternalOutput")

    GF = GJ * 1024
    with TileContext(nc) as tc:
        with (
            tc.tile_pool(name="wpool", bufs=1) as wpool,
            tc.tile_pool(name="xin", bufs=2) as xin_pool,
            tc.tile_pool(name="xsq", bufs=2) as xsq_pool,
            tc.tile_pool(name="yout", bufs=3) as y_pool,
            tc.tile_pool(name="small", bufs=4) as small_pool,
            tc.tile_pool(name="rep", bufs=3) as rep_pool,
            tc.tile_pool(name="psAB", bufs=3, space="PSUM") as psum_pool,
            tc.tile_pool(name="psS", bufs=1, space="PSUM") as psum_s_pool,
        ):
            wA_sb = wpool.tile([128, KCH * 97], F32R)
            wB_sb = wpool.tile([128, KCH * 96], F32R)
            negs_sb = wpool.tile([1, 192], F32R)
            tvec_sb = wpool.tile([1, 192], F32R)
            ones_sb = wpool.tile([1, 96], F32R)
            ebias = wpool.tile([1, 1], F32)
            nc.vector.memset(ebias[:], LN_EPS)
            nc.sync.dma_start(
                wA_sb[:].rearrange("p (q m) -> p q m", q=KCH),
                wA_d[:].rearrange("q p m -> p q m").bitcast(F32R))
            nc.sync.dma_start(
                wB_sb[:].rearrange("p (q m) -> p q m", q=KCH),
                wB_d[:].rearrange("q p m -> p q m").bitcast(F32R))
            nc.sync.dma_start(negs_sb[:], negs_d[:].bitcast(F32R))
            nc.sync.dma_start(tvec_sb[:], tvec_d[:].bitcast(F32R))
            nc.sync.dma_start(ones_sb[:], ones_d[:].bitcast(F32R))

            if reps > 1:
                import concourse.mybir as _mb
                loop_cm = tc.For_i(0, reps, 1,
                                   hint_engines=(_mb.EngineType.PE,
                                                 _mb.EngineType.SP,
                                                 _mb.EngineType.DVE,
                                                 _mb.EngineType.Activation))
            else:
                import contextlib
                loop_cm = contextlib.nullcontext()
            with loop_cm:
                for b in range(NB):
                    for dL in range(ND):
                        for jj in range(NJ // GJ):
                            X = xin_pool.tile([128, 3 * GF], F32R)
                            for dd in range(2):
                                for hh in range(2):
                                    p0 = dd * 64 + hh * 32
                                    for g in range(3):
                                        src = x[b, 32 * g:32 * g + 32,
                                                2 * dL + dd,
                                                16 * GJ * jj + hh:
                                                16 * GJ * (jj + 1): 2, :]
                                        src = src.bitcast(F32R)
                                        nc.sync.dma_start(
                                            X[p0:p0 + 32,
                                              g * GF:(g + 1) * GF], src)
                            for jl in range(GJ):
                                _tile_body(nc, y, b, dL, GJ * jj + jl, jl,
                                           X, GF,
                                           wA_sb, wB_sb, negs_sb, tvec_sb,
                                           ones_sb, ebias,
                                           xsq_pool, y_pool, small_pool,
                                           rep_pool, psum_pool, psum_s_pool,
                                           has_beta)
    nc.compile()
    return nc


_NC_CACHE = {}


def _get_nc(reps, has_beta):
    key = (reps, has_beta)
    if key not in _NC_CACHE:
        nc = bacc.Bacc("TRN2", target_bir_lowering=False)
        build_kernel(nc, reps=reps, has_beta=has_beta)
        _NC_CACHE[key] = nc
    return _NC_CACHE[key]


def _decode_y(y_raw):
    """(NB, ND, NJ, 96, 2, 512) -> (NB, 192, ND, 8*NJ, 64)"""
    z = y_raw.reshape(NB, ND, NJ, 96, 2, H2T, W2)
    z = z.transpose(0, 4, 3, 1, 2, 5, 6)
    return z.reshape(NB, 192, ND, NJ * H2T, W2)


def run_cores(x, w, gamma, beta, reps=1):
    """Run the SPMD kernel; returns full output (2, 192, 16, 64, 64)."""
    x = np.asarray(x, np.float32)
    prep, has_beta = _host_prep(w, gamma, beta)
    nc = _get_nc(reps, has_beta)
    in_maps = []
    for k in range(NCORES):
        m = {"x": np.ascontiguousarray(x[:, :, 4 * k:4 * k + 4, :, :])}
        m.update(prep)
        in_maps.append(m)
    res = run_bass_kernel_spmd(nc, in_maps, core_ids=list(range(NCORES)))
    out = np.empty((2, 192, 16, 64, 64), np.float32)
    for k in range(NCORES):
        out[:, :, 2 * k:2 * k + 2] = _decode_y(res.results[k]["y"])
    return out


def kernel(x, w, gamma, beta):
    return run_cores(x, w, gamma, beta, reps=1)



# revision 14
# speedup vs baseline: 6.5921x; 6.5921x over previous
"""nn_PatchMerging3D Trainium2 kernel (v2).

Full inputs: x (2, 96, 32, 128, 128) f32, w (192, 768), gamma (768), beta (768).
Output: (2, 192, 16, 64, 64) f32.

Sharding: D2 (=16) split across 8 cores, 2 d2-planes per core, both batches.
Per-core slab: x[:, :, 4k:4k+4, :, :].

Per-core kernel (bass/Tile):
  * SBUF X layout: partition p = dd*64 + hh*32 + c32 (c = 32g + c32),
    free = g*4096 + h2g*128 + w. 4 HWDGE DMAs per 4-tile group (one per
    (dd, hh), 1.5 MiB each, 4-D APs, 512B contiguous runs, disjoint
    32-partition quadrants).
  * LayerNorm folded into the 768x192 projection algebraically:
      wp = gamma*w ; w2 = wp - rowsum(wp)/768   (folds the -mu*s rank-1
      correction into the weights, so the PE never depends on stats)
      M2 = xf @ w2.T  (PE, 6 K=128 chunks, f32r, psA rows 0-95 + S1 ones
      row 96; psB rows 0-95), S2 = sum xf^2 via ACT Square + ones-lhsT
      matmuls into a 1-row psS bank.
    stats: ACT musq = Square(S1/768); DVE var = psS/768 - musq;
    ACT sig = sqrt(var + eps); DVE rsig = 1/sig; Pool(GpSimd)
    partition_broadcast reps rsig across 96 partitions; DVE scales
    psA/psB by rep into yt; 1 out-DMA per tile.
  * Emission is software-pipelined: group g+1's input DMAs are emitted
    before group g's tile bodies (the SP sequencer is in-order, so
    out-DMAs must not precede the next prefetch), and each tile emits
    the NEXT tile's ACT Square before its own stats ops (keeps the ACT
    queue from serializing squares behind latency-bound stats).
"""

import os
import sys

for _p in ("/opt/trn_rl_repo", "/root/.axon_site/_ro/trn_rl_repo"):
    if os.path.isdir(_p) and _p not in sys.path:
        sys.path.insert(0, _p)

import numpy as np

import concourse.bacc as bacc
import concourse.mybir as mybir
from concourse.tile import TileContext
from concourse.bass_utils import run_bass_kernel_spmd

F32 = mybir.dt.float32
F32R = mybir.dt.float32r
AF = mybir.ActivationFunctionType
OP = mybir.AluOpType

C = 96
W = 128
W2 = 64
H2T = 8          # h2 values per position tile
NPOS = 512       # positions per tile
KCH = 6          # contraction chunks (g, ww)
GJ = 4           # position tiles per DMA group
GF = GJ * 1024   # free extent of one g block in the X tile
LN_EPS = 1e-5
NCORES = 8
BCAST = "pool"   # "pool": GpSimd partition_broadcast; "pe": ones-matmul
MERGED_DMA = False  # 4-D DMA APs rejected by the AP balancer; use 12x 3-D

# full-problem per-core loop counts
NB, ND, NJ = 2, 2, 8


def _host_prep(w, gamma, beta):
    w = np.asarray(w, np.float32)
    gamma = np.asarray(gamma, np.float32)
    beta = np.asarray(beta, np.float32)
    wp = (w * gamma[None, :]).astype(np.float64)
    w2 = (wp - wp.sum(axis=1, keepdims=True) / 768.0).astype(np.float32)
    t = (w * beta[None, :]).sum(axis=1).astype(np.float32)

    dd = np.arange(2)[:, None, None]
    hh = np.arange(2)[None, :, None]
    ii = np.arange(32)[None, None, :]
    wA = np.zeros((KCH, 128, 97), np.float32)
    wB = np.zeros((KCH, 128, 96), np.float32)
    for g in range(3):
        for ww in range(2):
            q = g * 2 + ww
            cf = (dd * 384 + hh * 192 + ww * 96 + 32 * g + ii).reshape(128)
            wA[q, :, :96] = w2[0:96, cf].T
            wA[q, :, 96] = 1.0
            wB[q, :, :] = w2[96:192, cf].T
    tv = np.stack([t[0:96], t[96:192]], axis=1)  # [96, 2] per-partition adds
    return {
        "wA": wA,
        "wB": wB,
        "tvec": tv,
        "ones_row": np.ones((1, 96), np.float32),
    }, bool(np.any(beta != 0.0))


def _emit_group_loads(nc, x, xin_pool, b, dL, jj):
    """Emit the input DMAs for one 4-tile group; returns the X tile."""
    X = xin_pool.tile([128, 3 * GF], F32R)
    if MERGED_DMA:
        for dd in range(2):
            for hh in range(2):
                p0 = dd * 64 + hh * 32
                src = x[b, :, 2 * dL + dd,
                        16 * GJ * jj + hh: 16 * GJ * (jj + 1): 2, :]
                src = src.rearrange("(g c) h w -> c g h w", g=3)
                dst = X[p0:p0 + 32, :].rearrange(
                    "p (g h w) -> p g h w", g=3, h=16 * GJ // 2)
                nc.sync.dma_start(dst, src.bitcast(F32R))
    else:
        for dd in range(2):
            for hh in range(2):
                p0 = dd * 64 + hh * 32
                for g in range(3):
                    src = x[b, 32 * g:32 * g + 32, 2 * dL + dd,
                            16 * GJ * jj + hh: 16 * GJ * (jj + 1): 2, :]
                    nc.sync.dma_start(
                        X[p0:p0 + 32, g * GF:(g + 1) * GF],
                        src.bitcast(F32R))
    return X


def _emit_squares(nc, X, jl, xsq_pool):
    """One ACT Square covering all 3 g-chunks of tile jl; returns XQ."""
    h0 = H2T * jl
    XQ = xsq_pool.tile([128, 3072], F32R)
    src = X[:].rearrange("p (g h w) -> p g h w", g=3, h=8 * GJ)
    src = src[:, :, h0:h0 + H2T, :]
    dst = XQ[:].rearrange("p (g f) -> p g f", g=3)
    nc.scalar.activation(dst, src.bitcast(F32), AF.Square)
    return XQ


def _emit_out_dma(nc, y, out_rec):
    """Output DMA on the ACT HWDGE ring (qActDynamicHW): keeps the SP
    ring free for input prefetch (SP sequencer is in-order, and these
    DMAs wait on compute)."""
    yt_p, b_p, dL_p, j_p = out_rec
    nc.scalar.dma_start(
        y[b_p, dL_p, j_p],
        yt_p[:].rearrange("p (half f) -> p half f", half=2))


def _tile_body(nc, y, b, dL, j, jl, X, XQ,
               wA_sb, wB_sb, tv_sb, ones_sb, ebias,
               y_pool, small_pool, rep_pool, psum_pool, psum_s_pool,
               has_beta, prev_out):
    h0 = H2T * jl
    psA = psum_pool.tile([97, NPOS], F32)
    psB = psum_pool.tile([96, NPOS], F32)
    psS = psum_s_pool.tile([1, NPOS], F32)

    wA_v = wA_sb[:].rearrange("p (q m) -> p q m", q=KCH)
    wB_v = wB_sb[:].rearrange("p (q m) -> p q m", q=KCH)
    Xv = X[:].rearrange("p (g h2g w2 ww) -> p g h2g w2 ww",
                        g=3, h2g=H2T * GJ, w2=W2, ww=2)
    XQv = XQ[:].rearrange("p (g h2 w2 ww) -> p g h2 w2 ww",
                          g=3, h2=H2T, w2=W2, ww=2)
    # mains first (no XQ dependency), then the S2 row matmuls
    for g in range(3):
        for ww in range(2):
            q = g * 2 + ww
            rhs = Xv[:, g, h0:h0 + H2T, :, ww]
            nc.tensor.matmul(psA[:], wA_v[:, q], rhs,
                             start=(q == 0), stop=(q == KCH - 1))
            nc.tensor.matmul(psB[:], wB_v[:, q], rhs,
                             start=(q == 0), stop=(q == KCH - 1))
    for g in range(3):
        for ww in range(2):
            q = g * 2 + ww
            nc.tensor.matmul(psS[:], wA_v[:, q, 96:97],
                             XQv[:, g, :, :, ww],
                             start=(q == 0), stop=(q == KCH - 1))

    # stats: musq (ACT) -> var (DVE) -> sig (ACT) -> rsig (DVE)
    st = small_pool.tile([1, 4 * NPOS], F32)
    musq = st[:, 0:NPOS]
    var = st[:, NPOS:2 * NPOS]
    sig = st[:, 2 * NPOS:3 * NPOS]
    rsig = st[:, 3 * NPOS:]
    nc.scalar.activation(musq, psA[96:97, :], AF.Square, scale=1.0 / 768.0)
    nc.vector.scalar_tensor_tensor(var, psS[:], 1.0 / 768.0, musq,
                                   OP.mult, OP.subtract)
    nc.scalar.activation(sig, var, AF.Sqrt, bias=ebias[:])
    nc.vector.reciprocal(rsig, sig)

    # broadcast rsig across the 96 output partitions
    rep = rep_pool.tile([96, NPOS], F32)
    if BCAST == "pool":
        nc.gpsimd.partition_broadcast(rep[:], rsig)
    else:
        psR = psum_s_pool.tile([96, NPOS], F32)
        nc.tensor.matmul(psR[:], ones_sb[:], rsig.bitcast(F32R),
                         start=True, stop=True)
        nc.vector.tensor_copy(rep[:], psR[:])

    yt = y_pool.tile([96, 2 * NPOS], F32)
    nc.vector.tensor_tensor(yt[:, 0:NPOS], psA[0:96, :], rep[:], OP.mult)
    nc.vector.tensor_tensor(yt[:, NPOS:], psB[:], rep[:], OP.mult)
    if has_beta:
        nc.vector.tensor_scalar(yt[:, 0:NPOS], yt[:, 0:NPOS],
                                tv_sb[:, 0:1], None, OP.add)
        nc.vector.tensor_scalar(yt[:, NPOS:], yt[:, NPOS:],
                                tv_sb[:, 1:2], None, OP.add)
    return yt


def build_kernel(nc, reps=1, has_beta=True):
    x = nc.dram_tensor("x", [NB, C, 2 * ND, 16 * NJ, W], F32,
                       kind="ExternalInput")
    wA_d = nc.dram_tensor("wA", [KCH, 128, 97], F32, kind="ExternalInput")
    wB_d = nc.dram_tensor("wB", [KCH, 128, 96], F32, kind="ExternalInput")
    tvec_d = nc.dram_tensor("tvec", [96, 2], F32, kind="ExternalInput")
    ones_d = nc.dram_tensor("ones_row", [1, 96], F32, kind="ExternalInput")
    y = nc.dram_tensor("y", [NB, ND, NJ, 96, 2, NPOS], F32,
                       kind="ExternalOutput")

    with TileContext(nc) as tc:
        with (
            tc.tile_pool(name="wpool", bufs=1) as wpool,
            tc.tile_pool(name="xin", bufs=2) as xin_pool,
            tc.tile_pool(name="xsq", bufs=2) as xsq_pool,
            tc.tile_pool(name="yout", bufs=6) as y_pool,
            tc.tile_pool(name="small", bufs=3) as small_pool,
            tc.tile_pool(name="rep", bufs=3) as rep_pool,
            tc.tile_pool(name="psAB", bufs=3, space="PSUM") as psum_pool,
            tc.tile_pool(name="psS", bufs=2, space="PSUM") as psum_s_pool,
        ):
            wA_sb = wpool.tile([128, KCH * 97], F32R)
            wB_sb = wpool.tile([128, KCH * 96], F32R)
            tv_sb = wpool.tile([96, 2], F32)
            ones_sb = wpool.tile([1, 96], F32R)
            ebias = wpool.tile([1, 1], F32)
            nc.vector.memset(ebias[:], LN_EPS)
            nc.sync.dma_start(
                wA_sb[:].rearrange("p (q m) -> p q m", q=KCH),
                wA_d[:].rearrange("q p m -> p q m").bitcast(F32R))
            nc.sync.dma_start(
                wB_sb[:].rearrange("p (q m) -> p q m", q=KCH),
                wB_d[:].rearrange("q p m -> p q m").bitcast(F32R))
            nc.sync.dma_start(tv_sb[:], tvec_d[:])
            nc.sync.dma_start(ones_sb[:], ones_d[:].bitcast(F32R))

            if reps > 1:
                import concourse.mybir as _mb
                loop_cm = tc.For_i(0, reps, 1,
                                   hint_engines=(_mb.EngineType.PE,
                                                 _mb.EngineType.SP,
                                                 _mb.EngineType.DVE,
                                                 _mb.EngineType.Pool,
                                                 _mb.EngineType.Activation))
            else:
                import contextlib
                loop_cm = contextlib.nullcontext()
            with loop_cm:
                groups = [(b, dL, jj)
                          for b in range(NB)
                          for dL in range(ND)
                          for jj in range(NJ // GJ)]

                def flush_outs_sp(pending):
                    # out-DMAs deferred until after the next group's
                    # input prefetch is queued: the SP sequencer is
                    # in-order, so a compute-gated out-DMA ahead of a
                    # prefetch would stall the prefetch.  Same HWDGE
                    # ring as the inputs on purpose: a second ring
                    # round-robins at the DMA engines and delays the
                    # critical input transfers.
                    for yt_p, b_p, dL_p, j_p in pending:
                        nc.sync.dma_start(
                            y[b_p, dL_p, j_p],
                            yt_p[:].rearrange("p (half f) -> p half f",
                                              half=2))
                    pending.clear()

                X_cur = _emit_group_loads(nc, x, xin_pool, *groups[0])
                XQ_cur = None
                pending = []
                for gi, (b, dL, jj) in enumerate(groups):
                    last_group = gi + 1 >= len(groups)
                    if not last_group:
                        X_nxt = _emit_group_loads(nc, x, xin_pool,
                                                  *groups[gi + 1])
                    else:
                        X_nxt = None
                    flush_outs_sp(pending)
                    if XQ_cur is None:
                        XQ_cur = _emit_squares(nc, X_cur, 0, xsq_pool)
                    for jl in range(GJ):
                        # emit the NEXT tile's squares ahead of this
                        # tile's stats so the ACT queue stays busy
                        if jl + 1 < GJ:
                            XQ_nxt = _emit_squares(nc, X_cur, jl + 1,
                                                   xsq_pool)
                        elif X_nxt is not None:
                            XQ_nxt = _emit_squares(nc, X_nxt, 0, xsq_pool)
                        else:
                            XQ_nxt = None
                        yt = _tile_body(nc, y, b, dL, GJ * jj + jl, jl,
                                        X_cur, XQ_cur,
                                        wA_sb, wB_sb, tv_sb, ones_sb,
                                        ebias,
                                        y_pool, small_pool, rep_pool,
                                        psum_pool, psum_s_pool,
                                        has_beta, None)
                        pending.append((yt, b, dL, GJ * jj + jl))
                        XQ_cur = XQ_nxt
                    X_cur = X_nxt
                # the last group's outs go on the ACT ring: on the SP
                # ring they would gate the next rep's first prefetch
                # across the For_i back edge
                for rec in pending:
                    _emit_out_dma(nc, y, rec)
                pending.clear()
    nc.compile()
    return nc


_NC_CACHE = {}


def _get_nc(reps, has_beta):
    key = (reps, has_beta)
    if key not in _NC_CACHE:
        nc = bacc.Bacc("TRN2", target_bir_lowering=False)
        build_kernel(nc, reps=reps, has_beta=has_beta)
        _NC_CACHE[key] = nc
    return _NC_CACHE[key]


def _decode_y(y_raw):
    """(NB, ND, NJ, 96, 2, 512) -> (NB, 192, ND, 8*NJ, 64)"""
    z = y_raw.reshape(NB, ND, NJ, 96, 2, H2T, W2)
    z = z.transpose(0, 4, 3, 1, 2, 5, 6)
    return z.reshape(NB, 192, ND, NJ * H2T, W2)


def run_cores(x, w, gamma, beta, reps=1):
    """Run the SPMD kernel; returns full output (2, 192, 16, 64, 64)."""
    x = np.asarray(x, np.float32)
    prep, has_beta = _host_prep(w, gamma, beta)
    nc = _get_nc(reps, has_beta)
    in_maps = []
    for k in range(NCORES):
        m = {"x": np.ascontiguousarray(x[:, :, 4 * k:4 * k + 4, :, :])}
        m.update(prep)
        in_maps.append(m)
    res = run_bass_kernel_spmd(nc, in_maps, core_ids=list(range(NCORES)))
    out = np.empty((2, 192, 16, 64, 64), np.float32)
    for k in range(NCORES):
        out[:, :, 2 * k:2 * k + 2] = _decode_y(res.results[k]["y"])
    return out


def kernel(x, w, gamma, beta):
    return run_cores(x, w, gamma, beta, reps=1)
